# revision 37
# baseline (speedup 1.0000x reference)
"""Bass/Tile kernel for a single attention head, data-parallel over B=8 on
8 TRN2 NeuronCores (one batch element per core, no collectives).

Per-core problem (S=2048, D=1024, H=128):
    q = Xq @ Wq + bq ; k = Xk @ Wk + bk ; v = Xv @ Wv + bv
    out = softmax(q k^T / sqrt(H)) v

Layout strategy (PE contracts over the partition dim, so the contraction
operand must present d on partitions):
  - X^T [d, s] tiles built with PE (TensorEngine) transposes of the f32
    input tiles; the PSUM->SBUF drain casts to bf16 (so the bf16 cast is
    free - no separate cast pass, no DMA-transpose).
  - Projections produce q^T/k^T/v^T [d_out, s] (stationary W d-chunk
    bf16, moving X^T, N=512); the bias is a per-partition scalar in this
    layout and is fused into the ACT PSUM->SBUF drain.
  - Scores are computed transposed: scoresT [j, i] = k_j . q_i so the
    exp output feeds the AV matmul with no transpose. exp(x/sqrt(H)) is
    a single ACT pass PSUM->SBUF bf16 (scale folded into activation).
  - v is PE-transposed back to natural [s, H] and extended with a ones
    column; the AV matmul (stationary expT slice, moving [v|1], N=129)
    yields the output numerator AND the softmax row sums in the same
    PSUM accumulation. Normalization = DVE reciprocal + ACT copy with
    per-partition scale.
"""

import sys

if "/opt/trn_rl_repo" not in sys.path:
    sys.path.insert(0, "/opt/trn_rl_repo")

import numpy as np

import concourse.bass as bass
import concourse.tile as tile
from concourse import bacc, mybir
from concourse.bass_utils import run_bass_kernel_spmd
from concourse.masks import make_identity

P = 128          # partitions
S = 2048         # sequence length (per core)
D = 1024         # input dim
H = 128          # head dim (Dq = Dk)
ST = S // P      # 16 s-tiles
DC = D // P      # 8 d-chunks
NBLK = 512       # moving-operand block / PSUM quarter
NQ = S // NBLK   # 4 quarters
N_CORES = 8

F32 = mybir.dt.float32
BF16 = mybir.dt.bfloat16
AF = mybir.ActivationFunctionType

SOFTMAX_SCALE = 1.0 / float(np.sqrt(H))


def _build_kernel(tc, ins, out_ap):
    nc = tc.nc
    (q_in, k_in, v_in, Wq, bq, Wk, bk, Wv, bv) = ins

    with (
        tc.tile_pool(name="consts", bufs=1) as consts,
        tc.tile_pool(name="xraw", bufs=8) as rawp,
        tc.tile_pool(name="xt", bufs=2) as xtp,
        tc.tile_pool(name="proj", bufs=1) as projp,
        tc.tile_pool(name="vext", bufs=1) as vexp,
        tc.tile_pool(name="expp", bufs=1) as expp,
        tc.tile_pool(name="avout", bufs=4) as avoutp,
    ):
        # ---- identity for PE transposes (no DMA - keep ahead of loads) ----
        ident = consts.tile([P, P], F32, tag="ident")
        make_identity(nc, ident)
        ident_bf = consts.tile([P, P], BF16, tag="ident_bf")
        nc.vector.tensor_copy(ident_bf, ident)

        def load_consts():
            """Weights (cast to bf16) + biases.  Issued AFTER the first
            input's loads so the big DMA stream starts immediately."""
            w_tiles = []
            b_tiles = []
            for Wap, bap, nm in ((Wq, bq, "wq"), (Wk, bk, "wk"), (Wv, bv, "wv")):
                wf = consts.tile([P, DC, P], F32, tag=f"{nm}_f32")
                nc.sync.dma_start(
                    out=wf, in_=Wap.rearrange("(c p) m -> p c m", p=P)
                )
                wb = consts.tile([P, DC, P], BF16, tag=f"{nm}_bf")
                nc.vector.tensor_copy(wb, wf)
                bt = consts.tile([P, 1], F32, tag=f"{nm}_bias")
                nc.sync.dma_start(out=bt, in_=bap.rearrange("(p o) -> p o", o=1))
                w_tiles.append(wb)
                b_tiles.append(bt)
            return w_tiles, b_tiles

        # q^T / k^T as 4 independent quarter tiles: Tile tracks deps per
        # tile, so scores for k-quarter Q start as soon as that quarter
        # (and the q-quarter it reads) is drained - not after the whole
        # projection.
        qTq = [
            projp.tile([P, NBLK], BF16, tag=f"qT{i}", name=f"qT{i}")
            for i in range(NQ)
        ]
        kTq = [
            projp.tile([P, NBLK], BF16, tag=f"kT{i}", name=f"kT{i}")
            for i in range(NQ)
        ]
        vTq = [
            projp.tile([P, NBLK], BF16, tag=f"vT{i}", name=f"vT{i}")
            for i in range(NQ)
        ]
        expT = expp.tile([P, ST, S], BF16, tag="expT")
        # two v_ext tiles (j-tiles 0-7 / 8-15) so AV's early j-steps only
        # depend on the first half of v
        v_ext0 = vexp.tile([P, DC, H + 1], BF16, tag="v_ext0")
        v_ext1 = vexp.tile([P, DC, H + 1], BF16, tag="v_ext1")
        nc.gpsimd.memset(v_ext0[:, :, H : H + 1], 1.0)
        nc.gpsimd.memset(v_ext1[:, :, H : H + 1], 1.0)

        # PSUM budget (8 banks): psT 2x[128,128] (2) + psA 2x[128,512]
        # (2) + psS 2x[128,1024] (4) all live concurrently.
        with (
            tc.tile_pool(name="psT", bufs=2, space="PSUM") as psT,
            tc.tile_pool(name="psA", bufs=2, space="PSUM") as psA,
            tc.tile_pool(name="psS", bufs=2, space="PSUM") as psS,
        ):

            consts_loaded = []

            def input_pipeline(
                x_ap, widx, dst_bf, per_quarter=None, drain_act=False
            ):
                """Quarter-granular streaming: load 4 s-tiles (DMA, bf16
                straight from DRAM), PE-transpose (8 per s-tile batched
                into a 1-bank PSUM tile, one drain copy), project the
                quarter, then run the optional per-quarter continuation.

                Program order == dependency order so the Tile scheduler
                streams every stage behind the DMA.  drain_act routes the
                batched transpose-drain to ACT (a bool or per-quarter
                predicate) - used while ACT is idle pre-exp so DVE does
                not pace the pipeline.
                """
                XT = xtp.tile([P, DC, S], BF16, tag="xt")
                for nq in range(NQ):
                    use_act = drain_act(nq) if callable(drain_act) else drain_act
                    for st4 in range(4):
                        st = nq * 4 + st4
                        xr = rawp.tile([P, D], BF16, tag="xraw")
                        nc.sync.dma_start(
                            out=xr, in_=x_ap[st * P : (st + 1) * P, :]
                        )
                        if not consts_loaded:
                            consts_loaded.append(load_consts())
                        pst = psT.tile([P, DC, P], BF16, tag="pst")
                        for dc in range(DC):
                            nc.tensor.transpose(
                                pst[:, dc, :],
                                xr[:, dc * P : (dc + 1) * P],
                                ident_bf,
                            )
                        dst = XT[:, :, st * P : (st + 1) * P]
                        if use_act:
                            nc.scalar.copy(dst, pst)
                        else:
                            nc.vector.tensor_copy(dst, pst)
                    w_tiles, b_tiles = consts_loaded[0]
                    ps = psA.tile([P, NBLK], F32, tag="ps")
                    for dc in range(DC):
                        nc.tensor.matmul(
                            ps,
                            w_tiles[widx][:, dc, :],
                            XT[:, dc, nq * NBLK : (nq + 1) * NBLK],
                            start=(dc == 0),
                            stop=(dc == DC - 1),
                        )
                    # drain PSUM -> SBUF bf16 with the bias add fused;
                    # always on DVE so ACT's in-order queue stays free
                    # for casts and the exp stream
                    if isinstance(dst_bf, list):
                        dst = dst_bf[nq][:, :]
                    else:
                        dst = dst_bf[:, nq * NBLK : (nq + 1) * NBLK]
                    nc.vector.tensor_scalar_add(dst, ps, b_tiles[widx])
                    if per_quarter is not None:
                        per_quarter(nq)

            def scores_half(hf):
                # scoresT + exp for ALL 16 j-tiles, i-half hf. exp(jt,hf)
                # reads q quarters 2hf and 2hf+1 - run after qT[2hf+1].
                for jt in range(ST):
                    kt_sl = kTq[jt // 4][:, (jt % 4) * P : (jt % 4 + 1) * P]
                    pss = psS.tile([P, 1024], F32, tag="pss")
                    for nb in range(2):
                        nc.tensor.matmul(
                            pss[:, nb * NBLK : (nb + 1) * NBLK],
                            kt_sl,
                            qTq[2 * hf + nb][:, :],
                            start=True,
                            stop=True,
                        )
                    nc.scalar.activation(
                        expT[:, jt, hf * 1024 : (hf + 1) * 1024],
                        pss,
                        AF.Exp,
                        bias=0.0,
                        scale=SOFTMAX_SCALE,
                    )

            def q_quarter(nq):
                if nq == 1:
                    scores_half(0)
                elif nq == 3:
                    scores_half(1)

            # ---- load order k, q, v: every exp needs a PAIR of q
            # quarters plus all of k, so k first lets the exp stream
            # chase q's quarters; v is only needed by AV at the end ----
            input_pipeline(k_in, 1, kTq, drain_act=True)
            input_pipeline(
                q_in, 0, qTq, per_quarter=q_quarter,
                drain_act=lambda nq: nq < 2,
            )
            def v_quarterpair(nq):
                # after v quarters 0/1 (resp 2/3): transpose that half of
                # v back to natural layout [s, H] into its v_ext tile
                if nq not in (1, 3):
                    return
                jg = nq // 2
                vx = v_ext0 if jg == 0 else v_ext1
                psv = psT.tile([P, DC, P], BF16, tag="pst")
                for j in range(DC):
                    jt = jg * DC + j
                    nc.tensor.transpose(
                        psv[:, j, :],
                        vTq[jt // 4][:, (jt % 4) * P : (jt % 4 + 1) * P],
                        ident_bf,
                    )
                nc.vector.tensor_copy(vx[:, :, 0:P], psv)

            input_pipeline(v_in, 2, vTq, per_quarter=v_quarterpair)


        # ---- phase 3: AV + row sums in one accumulation, then normalize ----
        with tc.tile_pool(name="psB", bufs=4, space="PSUM") as psB:
            for it in range(ST):
                pso = psB.tile([P, H + 1], F32, tag="po")
                for jt in range(ST):
                    vx = v_ext0 if jt < DC else v_ext1
                    nc.tensor.matmul(
                        pso,
                        expT[:, jt, it * P : (it + 1) * P],
                        vx[:, jt % DC, :],
                        start=(jt == 0),
                        stop=(jt == ST - 1),
                    )
                rc = avoutp.tile([P, 1], F32, tag="recip")
                nc.vector.reciprocal(rc, pso[:, H : H + 1])
                ot = avoutp.tile([P, H], F32, tag="ot")
                nc.scalar.activation(ot, pso[:, 0:H], AF.Copy, bias=0.0, scale=rc)
                nc.sync.dma_start(out=out_ap[it * P : (it + 1) * P, :], in_=ot)


def build_nc():
    nc = bacc.Bacc(
        "TRN2", target_bir_lowering=False, debug=False, num_devices=N_CORES
    )
    names = ["query", "key", "value", "Wq", "bq", "Wk", "bk", "Wv", "bv"]
    shapes = {
        "query": [S, D],
        "key": [S, D],
        "value": [S, D],
        "Wq": [D, H],
        "bq": [H],
        "Wk": [D, H],
        "bk": [H],
        "Wv": [D, H],
        "bv": [H],
    }
    # query/key/value land in DRAM as bf16 (host-cast in _run): the
    # kernel computes in bf16 anyway and this halves the HBM traffic
    dtypes = {n: (BF16 if n in ("query", "key", "value") else F32) for n in names}
    ins = [
        nc.dram_tensor(n, shapes[n], dtypes[n], kind="ExternalInput").ap()
        for n in names
    ]
    out_ap = nc.dram_tensor("out", [S, H], F32, kind="ExternalOutput").ap()
    with tile.TileContext(nc) as tc:
        _build_kernel(tc, ins, out_ap)
    nc.compile()
    return nc


_NC_CACHE = None


def _get_nc():
    global _NC_CACHE
    if _NC_CACHE is None:
        _NC_CACHE = build_nc()
    return _NC_CACHE


def _run(inputs, trace=False, **kw):
    import ml_dtypes

    nc = _get_nc()
    bf = np.dtype(ml_dtypes.bfloat16)
    qf = np.ascontiguousarray(
        np.asarray(inputs["query"], dtype=np.float32).astype(bf)
    )
    kf = np.ascontiguousarray(
        np.asarray(inputs["key"], dtype=np.float32).astype(bf)
    )
    vf = np.ascontiguousarray(
        np.asarray(inputs["value"], dtype=np.float32).astype(bf)
    )
    shared = {
        n: np.ascontiguousarray(np.asarray(inputs[n], dtype=np.float32))
        for n in ["Wq", "bq", "Wk", "bk", "Wv", "bv"]
    }
    in_maps = [
        {"query": qf[c], "key": kf[c], "value": vf[c], **shared}
        for c in range(N_CORES)
    ]
    res = run_bass_kernel_spmd(nc, in_maps, list(range(N_CORES)), trace=trace, **kw)
    out = np.stack([res.results[c]["out"] for c in range(N_CORES)], axis=0)
    return out.astype(np.float32), res


def kernel(**inputs) -> np.ndarray:
    out, _ = _run(inputs, trace=False)
    return out


if __name__ == "__main__":
    # smoke-build only
    build_nc()
    print("build ok")


# revision 40
# speedup vs baseline: 1.0651x; 1.0651x over previous
"""Bass/Tile kernel for a single attention head, data-parallel over B=8 on
8 TRN2 NeuronCores (one batch element per core, no collectives).

Per-core problem (S=2048, D=1024, H=128):
    q = Xq @ Wq + bq ; k = Xk @ Wk + bk ; v = Xv @ Wv + bv
    out = softmax(q k^T / sqrt(H)) v

Layout strategy (PE contracts over the partition dim, so the contraction
operand must present d on partitions):
  - X^T [d, s] tiles built with PE (TensorEngine) transposes of the f32
    input tiles; the PSUM->SBUF drain casts to bf16 (so the bf16 cast is
    free - no separate cast pass, no DMA-transpose).
  - Projections produce q^T/k^T/v^T [d_out, s] (stationary W d-chunk
    bf16, moving X^T, N=512); the bias is a per-partition scalar in this
    layout and is fused into the ACT PSUM->SBUF drain.
  - Scores are computed transposed: scoresT [j, i] = k_j . q_i so the
    exp output feeds the AV matmul with no transpose. exp(x/sqrt(H)) is
    a single ACT pass PSUM->SBUF bf16 (scale folded into activation).
  - v is PE-transposed back to natural [s, H] and extended with a ones
    column; the AV matmul (stationary expT slice, moving [v|1], N=129)
    yields the output numerator AND the softmax row sums in the same
    PSUM accumulation. Normalization = DVE reciprocal + ACT copy with
    per-partition scale.
"""

import sys

if "/opt/trn_rl_repo" not in sys.path:
    sys.path.insert(0, "/opt/trn_rl_repo")

import numpy as np

import concourse.bass as bass
import concourse.tile as tile
from concourse import bacc, mybir
from concourse.bass_utils import run_bass_kernel_spmd
from concourse.masks import make_identity

P = 128          # partitions
S = 2048         # sequence length (per core)
D = 1024         # input dim
H = 128          # head dim (Dq = Dk)
ST = S // P      # 16 s-tiles
DC = D // P      # 8 d-chunks
NBLK = 512       # moving-operand block / PSUM quarter
NQ = S // NBLK   # 4 quarters
N_CORES = 8

F32 = mybir.dt.float32
BF16 = mybir.dt.bfloat16
AF = mybir.ActivationFunctionType

SOFTMAX_SCALE = 1.0 / float(np.sqrt(H))


def _build_kernel(tc, ins, out_ap):
    nc = tc.nc
    (q_in, k_in, v_in, Wq, bq, Wk, bk, Wv, bv) = ins

    with (
        tc.tile_pool(name="consts", bufs=1) as consts,
        tc.tile_pool(name="xraw", bufs=8) as rawp,
        tc.tile_pool(name="xt", bufs=2) as xtp,
        tc.tile_pool(name="proj", bufs=1) as projp,
        tc.tile_pool(name="vext", bufs=1) as vexp,
        tc.tile_pool(name="expp", bufs=1) as expp,
        tc.tile_pool(name="avout", bufs=4) as avoutp,
    ):
        # ---- identity for PE transposes (no DMA - keep ahead of loads) ----
        ident = consts.tile([P, P], F32, tag="ident")
        make_identity(nc, ident)
        ident_bf = consts.tile([P, P], BF16, tag="ident_bf")
        nc.vector.tensor_copy(ident_bf, ident)
        warm_sink = nc.dram_tensor("warm_sink", [P, P], F32)

        def load_consts():
            """Weights (cast to bf16) + biases.  Issued AFTER the first
            input's loads so the big DMA stream starts immediately."""
            w_tiles = []
            b_tiles = []
            for Wap, bap, nm in ((Wq, bq, "wq"), (Wk, bk, "wk"), (Wv, bv, "wv")):
                wf = consts.tile([P, DC, P], F32, tag=f"{nm}_f32")
                nc.sync.dma_start(
                    out=wf, in_=Wap.rearrange("(c p) m -> p c m", p=P)
                )
                wb = consts.tile([P, DC, P], BF16, tag=f"{nm}_bf")
                nc.vector.tensor_copy(wb, wf)
                bt = consts.tile([P, 1], F32, tag=f"{nm}_bias")
                nc.sync.dma_start(out=bt, in_=bap.rearrange("(p o) -> p o", o=1))
                w_tiles.append(wb)
                b_tiles.append(bt)
            return w_tiles, b_tiles

        # q^T / k^T as 4 independent quarter tiles: Tile tracks deps per
        # tile, so scores for k-quarter Q start as soon as that quarter
        # (and the q-quarter it reads) is drained - not after the whole
        # projection.
        qTq = [
            projp.tile([P, NBLK], BF16, tag=f"qT{i}", name=f"qT{i}")
            for i in range(NQ)
        ]
        kTq = [
            projp.tile([P, NBLK], BF16, tag=f"kT{i}", name=f"kT{i}")
            for i in range(NQ)
        ]
        vTq = [
            projp.tile([P, NBLK], BF16, tag=f"vT{i}", name=f"vT{i}")
            for i in range(NQ)
        ]
        expT = expp.tile([P, ST, S], BF16, tag="expT")
        # two v_ext tiles (j-tiles 0-7 / 8-15) so AV's early j-steps only
        # depend on the first half of v
        v_ext0 = vexp.tile([P, DC, H + 1], BF16, tag="v_ext0")
        v_ext1 = vexp.tile([P, DC, H + 1], BF16, tag="v_ext1")
        nc.gpsimd.memset(v_ext0[:, :, H : H + 1], 1.0)
        nc.gpsimd.memset(v_ext1[:, :, H : H + 1], 1.0)

        # PSUM budget (8 banks): psT 2x[128,128] (2) + psA 2x[128,512]
        # (2) + psS 2x[128,1024] (4) all live concurrently.
        with (
            tc.tile_pool(name="psT", bufs=2, space="PSUM") as psT,
            tc.tile_pool(name="psA", bufs=2, space="PSUM") as psA,
            tc.tile_pool(name="psS", bufs=2, space="PSUM") as psS,
        ):

            consts_loaded = []

            def input_pipeline(
                x_ap, widx, dst_bf, per_quarter=None, drain_act=False
            ):
                """Quarter-granular streaming: load 4 s-tiles (DMA, bf16
                straight from DRAM), PE-transpose (8 per s-tile batched
                into a 1-bank PSUM tile, one drain copy), project the
                quarter, then run the optional per-quarter continuation.

                Program order == dependency order so the Tile scheduler
                streams every stage behind the DMA.  drain_act routes the
                batched transpose-drain to ACT (a bool or per-quarter
                predicate) - used while ACT is idle pre-exp so DVE does
                not pace the pipeline.
                """
                XT = xtp.tile([P, DC, S], BF16, tag="xt")
                for nq in range(NQ):
                    use_act = drain_act(nq) if callable(drain_act) else drain_act
                    for st4 in range(4):
                        st = nq * 4 + st4
                        xr = rawp.tile([P, D], BF16, tag="xraw")
                        nc.sync.dma_start(
                            out=xr, in_=x_ap[st * P : (st + 1) * P, :]
                        )
                        if not consts_loaded:
                            consts_loaded.append(load_consts())
                        pst = psT.tile([P, DC, P], BF16, tag="pst")
                        for dc in range(DC):
                            nc.tensor.transpose(
                                pst[:, dc, :],
                                xr[:, dc * P : (dc + 1) * P],
                                ident_bf,
                            )
                        dst = XT[:, :, st * P : (st + 1) * P]
                        if use_act:
                            nc.scalar.copy(dst, pst)
                        else:
                            nc.vector.tensor_copy(dst, pst)
                    w_tiles, b_tiles = consts_loaded[0]
                    ps = psA.tile([P, NBLK], F32, tag="ps")
                    for dc in range(DC):
                        nc.tensor.matmul(
                            ps,
                            w_tiles[widx][:, dc, :],
                            XT[:, dc, nq * NBLK : (nq + 1) * NBLK],
                            start=(dc == 0),
                            stop=(dc == DC - 1),
                        )
                    # drain PSUM -> SBUF bf16 with the bias add fused;
                    # always on DVE so ACT's in-order queue stays free
                    # for casts and the exp stream
                    if isinstance(dst_bf, list):
                        dst = dst_bf[nq][:, :]
                    else:
                        dst = dst_bf[:, nq * NBLK : (nq + 1) * NBLK]
                    nc.vector.tensor_scalar_add(dst, ps, b_tiles[widx])
                    if per_quarter is not None:
                        per_quarter(nq)

            def scores_half(hf):
                # scoresT + exp for ALL 16 j-tiles, i-half hf. exp(jt,hf)
                # reads q quarters 2hf and 2hf+1 - run after qT[2hf+1].
                for jt in range(ST):
                    kt_sl = kTq[jt // 4][:, (jt % 4) * P : (jt % 4 + 1) * P]
                    pss = psS.tile([P, 1024], F32, tag="pss")
                    for nb in range(2):
                        nc.tensor.matmul(
                            pss[:, nb * NBLK : (nb + 1) * NBLK],
                            kt_sl,
                            qTq[2 * hf + nb][:, :],
                            start=True,
                            stop=True,
                        )
                    nc.scalar.activation(
                        expT[:, jt, hf * 1024 : (hf + 1) * 1024],
                        pss,
                        AF.Exp,
                        bias=0.0,
                        scale=SOFTMAX_SCALE,
                    )

            def q_quarter(nq):
                if nq == 1:
                    scores_half(0)
                elif nq == 3:
                    scores_half(1)

            # ---- PE warm-up: the HAM clock gate keeps the PE at 1.2GHz
            # until ~3.4us of sustained activity.  Burn dummy matmuls on
            # the identity during the initial DMA dead-time so the real
            # transpose stream runs at 2.4GHz from the start.  The result
            # is DMA'd to a DRAM sink so the chain is not dead code. ----
            ps_warm = psT.tile([P, P], F32, tag="pst", name="ps_warm")
            warm_sb = consts.tile([P, P], F32, tag="warm_sb")
            for _ in range(120):
                nc.tensor.matmul(ps_warm, ident_bf, ident_bf, start=True, stop=True)
            nc.vector.tensor_copy(warm_sb, ps_warm)
            nc.sync.dma_start(out=warm_sink[:, :], in_=warm_sb)

            # ---- load order k, q, v: every exp needs a PAIR of q
            # quarters plus all of k, so k first lets the exp stream
            # chase q's quarters; v is only needed by AV at the end ----
            input_pipeline(k_in, 1, kTq)
            input_pipeline(q_in, 0, qTq, per_quarter=q_quarter)
            def v_quarterpair(nq):
                # after v quarters 0/1 (resp 2/3): transpose that half of
                # v back to natural layout [s, H] into its v_ext tile
                if nq not in (1, 3):
                    return
                jg = nq // 2
                vx = v_ext0 if jg == 0 else v_ext1
                psv = psT.tile([P, DC, P], BF16, tag="pst")
                for j in range(DC):
                    jt = jg * DC + j
                    nc.tensor.transpose(
                        psv[:, j, :],
                        vTq[jt // 4][:, (jt % 4) * P : (jt % 4 + 1) * P],
                        ident_bf,
                    )
                nc.vector.tensor_copy(vx[:, :, 0:P], psv)

            input_pipeline(v_in, 2, vTq, per_quarter=v_quarterpair)


        # ---- phase 3: AV + row sums in one accumulation, then normalize ----
        with tc.tile_pool(name="psB", bufs=4, space="PSUM") as psB:
            for it in range(ST):
                pso = psB.tile([P, H + 1], F32, tag="po")
                for jt in range(ST):
                    vx = v_ext0 if jt < DC else v_ext1
                    nc.tensor.matmul(
                        pso,
                        expT[:, jt, it * P : (it + 1) * P],
                        vx[:, jt % DC, :],
                        start=(jt == 0),
                        stop=(jt == ST - 1),
                    )
                rc = avoutp.tile([P, 1], F32, tag="recip")
                nc.vector.reciprocal(rc, pso[:, H : H + 1])
                ot = avoutp.tile([P, H], F32, tag="ot")
                nc.scalar.activation(ot, pso[:, 0:H], AF.Copy, bias=0.0, scale=rc)
                nc.sync.dma_start(out=out_ap[it * P : (it + 1) * P, :], in_=ot)


def build_nc():
    nc = bacc.Bacc(
        "TRN2", target_bir_lowering=False, debug=False, num_devices=N_CORES
    )
    names = ["query", "key", "value", "Wq", "bq", "Wk", "bk", "Wv", "bv"]
    shapes = {
        "query": [S, D],
        "key": [S, D],
        "value": [S, D],
        "Wq": [D, H],
        "bq": [H],
        "Wk": [D, H],
        "bk": [H],
        "Wv": [D, H],
        "bv": [H],
    }
    # query/key/value land in DRAM as bf16 (host-cast in _run): the
    # kernel computes in bf16 anyway and this halves the HBM traffic
    dtypes = {n: (BF16 if n in ("query", "key", "value") else F32) for n in names}
    ins = [
        nc.dram_tensor(n, shapes[n], dtypes[n], kind="ExternalInput").ap()
        for n in names
    ]
    out_ap = nc.dram_tensor("out", [S, H], F32, kind="ExternalOutput").ap()
    with tile.TileContext(nc) as tc:
        _build_kernel(tc, ins, out_ap)
    nc.compile()
    return nc


_NC_CACHE = None


def _get_nc():
    global _NC_CACHE
    if _NC_CACHE is None:
        _NC_CACHE = build_nc()
    return _NC_CACHE


def _run(inputs, trace=False, **kw):
    import ml_dtypes

    nc = _get_nc()
    bf = np.dtype(ml_dtypes.bfloat16)
    qf = np.ascontiguousarray(
        np.asarray(inputs["query"], dtype=np.float32).astype(bf)
    )
    kf = np.ascontiguousarray(
        np.asarray(inputs["key"], dtype=np.float32).astype(bf)
    )
    vf = np.ascontiguousarray(
        np.asarray(inputs["value"], dtype=np.float32).astype(bf)
    )
    shared = {
        n: np.ascontiguousarray(np.asarray(inputs[n], dtype=np.float32))
        for n in ["Wq", "bq", "Wk", "bk", "Wv", "bv"]
    }
    in_maps = [
        {"query": qf[c], "key": kf[c], "value": vf[c], **shared}
        for c in range(N_CORES)
    ]
    res = run_bass_kernel_spmd(nc, in_maps, list(range(N_CORES)), trace=trace, **kw)
    out = np.stack([res.results[c]["out"] for c in range(N_CORES)], axis=0)
    return out.astype(np.float32), res


def kernel(**inputs) -> np.ndarray:
    out, _ = _run(inputs, trace=False)
    return out


if __name__ == "__main__":
    # smoke-build only
    build_nc()
    print("build ok")


# revision 42
# speedup vs baseline: 1.0667x; 1.0015x over previous
"""Bass/Tile kernel for a single attention head, data-parallel over B=8 on
8 TRN2 NeuronCores (one batch element per core, no collectives).

Per-core problem (S=2048, D=1024, H=128):
    q = Xq @ Wq + bq ; k = Xk @ Wk + bk ; v = Xv @ Wv + bv
    out = softmax(q k^T / sqrt(H)) v

Layout strategy (PE contracts over the partition dim, so the contraction
operand must present d on partitions):
  - X^T [d, s] tiles built with PE (TensorEngine) transposes of the f32
    input tiles; the PSUM->SBUF drain casts to bf16 (so the bf16 cast is
    free - no separate cast pass, no DMA-transpose).
  - Projections produce q^T/k^T/v^T [d_out, s] (stationary W d-chunk
    bf16, moving X^T, N=512); the bias is a per-partition scalar in this
    layout and is fused into the ACT PSUM->SBUF drain.
  - Scores are computed transposed: scoresT [j, i] = k_j . q_i so the
    exp output feeds the AV matmul with no transpose. exp(x/sqrt(H)) is
    a single ACT pass PSUM->SBUF bf16 (scale folded into activation).
  - v is PE-transposed back to natural [s, H] and extended with a ones
    column; the AV matmul (stationary expT slice, moving [v|1], N=129)
    yields the output numerator AND the softmax row sums in the same
    PSUM accumulation. Normalization = DVE reciprocal + ACT copy with
    per-partition scale.
"""

import sys

if "/opt/trn_rl_repo" not in sys.path:
    sys.path.insert(0, "/opt/trn_rl_repo")

import numpy as np

import concourse.bass as bass
import concourse.tile as tile
from concourse import bacc, mybir
from concourse.bass_utils import run_bass_kernel_spmd
from concourse.masks import make_identity

P = 128          # partitions
S = 2048         # sequence length (per core)
D = 1024         # input dim
H = 128          # head dim (Dq = Dk)
ST = S // P      # 16 s-tiles
DC = D // P      # 8 d-chunks
NBLK = 512       # moving-operand block / PSUM quarter
NQ = S // NBLK   # 4 quarters
N_CORES = 8

F32 = mybir.dt.float32
BF16 = mybir.dt.bfloat16
AF = mybir.ActivationFunctionType

SOFTMAX_SCALE = 1.0 / float(np.sqrt(H))


def _build_kernel(tc, ins, out_ap):
    nc = tc.nc
    (q_in, k_in, v_in, Wq, bq, Wk, bk, Wv, bv) = ins

    with (
        tc.tile_pool(name="consts", bufs=1) as consts,
        tc.tile_pool(name="xraw", bufs=8) as rawp,
        tc.tile_pool(name="xt", bufs=2) as xtp,
        tc.tile_pool(name="proj", bufs=1) as projp,
        tc.tile_pool(name="vext", bufs=1) as vexp,
        tc.tile_pool(name="expp", bufs=1) as expp,
        tc.tile_pool(name="avout", bufs=4) as avoutp,
    ):
        # ---- identity for PE transposes (no DMA - keep ahead of loads) ----
        ident = consts.tile([P, P], F32, tag="ident")
        make_identity(nc, ident)
        ident_bf = consts.tile([P, P], BF16, tag="ident_bf")
        nc.vector.tensor_copy(ident_bf, ident)
        warm_sink = nc.dram_tensor("warm_sink", [P, P], F32)

        def load_consts():
            """Weights (cast to bf16) + biases.  Issued AFTER the first
            input's loads so the big DMA stream starts immediately."""
            w_tiles = []
            b_tiles = []
            for Wap, bap, nm in ((Wq, bq, "wq"), (Wk, bk, "wk"), (Wv, bv, "wv")):
                wf = consts.tile([P, DC, P], F32, tag=f"{nm}_f32")
                nc.sync.dma_start(
                    out=wf, in_=Wap.rearrange("(c p) m -> p c m", p=P)
                )
                wb = consts.tile([P, DC, P], BF16, tag=f"{nm}_bf")
                nc.vector.tensor_copy(wb, wf)
                bt = consts.tile([P, 1], F32, tag=f"{nm}_bias")
                nc.sync.dma_start(out=bt, in_=bap.rearrange("(p o) -> p o", o=1))
                w_tiles.append(wb)
                b_tiles.append(bt)
            return w_tiles, b_tiles

        # q^T / k^T as 4 independent quarter tiles: Tile tracks deps per
        # tile, so scores for k-quarter Q start as soon as that quarter
        # (and the q-quarter it reads) is drained - not after the whole
        # projection.
        qTq = [
            projp.tile([P, NBLK], BF16, tag=f"qT{i}", name=f"qT{i}")
            for i in range(NQ)
        ]
        kTq = [
            projp.tile([P, NBLK], BF16, tag=f"kT{i}", name=f"kT{i}")
            for i in range(NQ)
        ]
        vTq = [
            projp.tile([P, NBLK], BF16, tag=f"vT{i}", name=f"vT{i}")
            for i in range(NQ)
        ]
        expT = expp.tile([P, ST, S], BF16, tag="expT")
        # two v_ext tiles (j-tiles 0-7 / 8-15) so AV's early j-steps only
        # depend on the first half of v
        v_ext0 = vexp.tile([P, DC, H + 1], BF16, tag="v_ext0")
        v_ext1 = vexp.tile([P, DC, H + 1], BF16, tag="v_ext1")
        nc.gpsimd.memset(v_ext0[:, :, H : H + 1], 1.0)
        nc.gpsimd.memset(v_ext1[:, :, H : H + 1], 1.0)

        # PSUM budget (8 banks): psT 2x[128,128] (2) + psA 2x[128,512]
        # (2) + psS 2x[128,1024] (4) all live concurrently.
        with (
            tc.tile_pool(name="psT", bufs=2, space="PSUM") as psT,
            tc.tile_pool(name="psA", bufs=2, space="PSUM") as psA,
            tc.tile_pool(name="psS", bufs=2, space="PSUM") as psS,
        ):

            consts_loaded = []

            def input_pipeline(
                x_ap, widx, dst_bf, per_quarter=None, drain_act=False
            ):
                """Quarter-granular streaming: load 4 s-tiles (DMA, bf16
                straight from DRAM), PE-transpose (8 per s-tile batched
                into a 1-bank PSUM tile, one drain copy), project the
                quarter, then run the optional per-quarter continuation.

                Program order == dependency order so the Tile scheduler
                streams every stage behind the DMA.  drain_act routes the
                batched transpose-drain to ACT (a bool or per-quarter
                predicate) - used while ACT is idle pre-exp so DVE does
                not pace the pipeline.
                """
                XT = xtp.tile([P, DC, S], BF16, tag="xt")
                for nq in range(NQ):
                    use_act = drain_act(nq) if callable(drain_act) else drain_act
                    for st4 in range(4):
                        st = nq * 4 + st4
                        xr = rawp.tile([P, D], BF16, tag="xraw")
                        nc.sync.dma_start(
                            out=xr, in_=x_ap[st * P : (st + 1) * P, :]
                        )
                        if not consts_loaded:
                            consts_loaded.append(load_consts())
                        pst = psT.tile([P, DC, P], BF16, tag="pst")
                        for dc in range(DC):
                            nc.tensor.transpose(
                                pst[:, dc, :],
                                xr[:, dc * P : (dc + 1) * P],
                                ident_bf,
                            )
                        dst = XT[:, :, st * P : (st + 1) * P]
                        if use_act:
                            nc.scalar.copy(dst, pst)
                        else:
                            nc.vector.tensor_copy(dst, pst)
                    w_tiles, b_tiles = consts_loaded[0]
                    ps = psA.tile([P, NBLK], F32, tag="ps")
                    for dc in range(DC):
                        nc.tensor.matmul(
                            ps,
                            w_tiles[widx][:, dc, :],
                            XT[:, dc, nq * NBLK : (nq + 1) * NBLK],
                            start=(dc == 0),
                            stop=(dc == DC - 1),
                        )
                    # drain PSUM -> SBUF bf16 with the bias add fused;
                    # always on DVE so ACT's in-order queue stays free
                    # for casts and the exp stream
                    if isinstance(dst_bf, list):
                        dst = dst_bf[nq][:, :]
                    else:
                        dst = dst_bf[:, nq * NBLK : (nq + 1) * NBLK]
                    nc.vector.tensor_scalar_add(dst, ps, b_tiles[widx])
                    if per_quarter is not None:
                        per_quarter(nq)

            def scores_half(hf):
                # scoresT + exp for ALL 16 j-tiles, i-half hf. exp(jt,hf)
                # reads q quarters 2hf and 2hf+1 - run after qT[2hf+1].
                for jt in range(ST):
                    kt_sl = kTq[jt // 4][:, (jt % 4) * P : (jt % 4 + 1) * P]
                    pss = psS.tile([P, 1024], F32, tag="pss")
                    for nb in range(2):
                        nc.tensor.matmul(
                            pss[:, nb * NBLK : (nb + 1) * NBLK],
                            kt_sl,
                            qTq[2 * hf + nb][:, :],
                            start=True,
                            stop=True,
                        )
                    nc.scalar.activation(
                        expT[:, jt, hf * 1024 : (hf + 1) * 1024],
                        pss,
                        AF.Exp,
                        bias=0.0,
                        scale=SOFTMAX_SCALE,
                    )

            def q_quarter(nq):
                if nq == 1:
                    scores_half(0)
                elif nq == 3:
                    scores_half(1)

            # ---- PE warm-up: the HAM clock gate keeps the PE at 1.2GHz
            # until ~3.4us of sustained activity.  Burn dummy matmuls on
            # the identity during the initial DMA dead-time so the real
            # transpose stream runs at 2.4GHz from the start.  The result
            # is DMA'd to a DRAM sink so the chain is not dead code. ----
            ps_warm = psT.tile([P, P], F32, tag="pst", name="ps_warm")
            warm_sb = consts.tile([P, P], F32, tag="warm_sb")
            for _ in range(120):
                nc.tensor.matmul(ps_warm, ident_bf, ident_bf, start=True, stop=True)
            nc.vector.tensor_copy(warm_sb, ps_warm)
            nc.sync.dma_start(out=warm_sink[:, :], in_=warm_sb)

            # ---- load order k, q, v: every exp needs a PAIR of q
            # quarters plus all of k, so k first lets the exp stream
            # chase q's quarters; v is only needed by AV at the end ----
            input_pipeline(k_in, 1, kTq)
            input_pipeline(q_in, 0, qTq, per_quarter=q_quarter)
            def v_quarterpair(nq):
                # after v quarters 0/1 (resp 2/3): transpose that half of
                # v back to natural layout [s, H] into its v_ext tile
                if nq not in (1, 3):
                    return
                jg = nq // 2
                vx = v_ext0 if jg == 0 else v_ext1
                psv = psT.tile([P, DC, P], BF16, tag="pst")
                for j in range(DC):
                    jt = jg * DC + j
                    nc.tensor.transpose(
                        psv[:, j, :],
                        vTq[jt // 4][:, (jt % 4) * P : (jt % 4 + 1) * P],
                        ident_bf,
                    )
                nc.vector.tensor_copy(vx[:, :, 0:P], psv)

            input_pipeline(v_in, 2, vTq, per_quarter=v_quarterpair)


        # ---- phase 3: AV + row sums in one accumulation, then normalize ----
        with tc.tile_pool(name="psB", bufs=6, space="PSUM") as psB:
            for it in range(ST):
                pso = psB.tile([P, H + 1], F32, tag="po")
                for jt in range(ST):
                    vx = v_ext0 if jt < DC else v_ext1
                    nc.tensor.matmul(
                        pso,
                        expT[:, jt, it * P : (it + 1) * P],
                        vx[:, jt % DC, :],
                        start=(jt == 0),
                        stop=(jt == ST - 1),
                    )
                rc = avoutp.tile([P, 1], F32, tag="recip")
                nc.vector.reciprocal(rc, pso[:, H : H + 1])
                ot = avoutp.tile([P, H], F32, tag="ot")
                nc.vector.tensor_scalar_mul(ot, pso[:, 0:H], rc)
                nc.sync.dma_start(out=out_ap[it * P : (it + 1) * P, :], in_=ot)


def build_nc():
    nc = bacc.Bacc(
        "TRN2", target_bir_lowering=False, debug=False, num_devices=N_CORES
    )
    names = ["query", "key", "value", "Wq", "bq", "Wk", "bk", "Wv", "bv"]
    shapes = {
        "query": [S, D],
        "key": [S, D],
        "value": [S, D],
        "Wq": [D, H],
        "bq": [H],
        "Wk": [D, H],
        "bk": [H],
        "Wv": [D, H],
        "bv": [H],
    }
    # query/key/value land in DRAM as bf16 (host-cast in _run): the
    # kernel computes in bf16 anyway and this halves the HBM traffic
    dtypes = {n: (BF16 if n in ("query", "key", "value") else F32) for n in names}
    ins = [
        nc.dram_tensor(n, shapes[n], dtypes[n], kind="ExternalInput").ap()
        for n in names
    ]
    out_ap = nc.dram_tensor("out", [S, H], F32, kind="ExternalOutput").ap()
    with tile.TileContext(nc) as tc:
        _build_kernel(tc, ins, out_ap)
    nc.compile()
    return nc


_NC_CACHE = None


def _get_nc():
    global _NC_CACHE
    if _NC_CACHE is None:
        _NC_CACHE = build_nc()
    return _NC_CACHE


def _run(inputs, trace=False, **kw):
    import ml_dtypes

    nc = _get_nc()
    bf = np.dtype(ml_dtypes.bfloat16)
    qf = np.ascontiguousarray(
        np.asarray(inputs["query"], dtype=np.float32).astype(bf)
    )
    kf = np.ascontiguousarray(
        np.asarray(inputs["key"], dtype=np.float32).astype(bf)
    )
    vf = np.ascontiguousarray(
        np.asarray(inputs["value"], dtype=np.float32).astype(bf)
    )
    shared = {
        n: np.ascontiguousarray(np.asarray(inputs[n], dtype=np.float32))
        for n in ["Wq", "bq", "Wk", "bk", "Wv", "bv"]
    }
    in_maps = [
        {"query": qf[c], "key": kf[c], "value": vf[c], **shared}
        for c in range(N_CORES)
    ]
    res = run_bass_kernel_spmd(nc, in_maps, list(range(N_CORES)), trace=trace, **kw)
    out = np.stack([res.results[c]["out"] for c in range(N_CORES)], axis=0)
    return out.astype(np.float32), res


def kernel(**inputs) -> np.ndarray:
    out, _ = _run(inputs, trace=False)
    return out


if __name__ == "__main__":
    # smoke-build only
    build_nc()
    print("build ok")
